# revision 1
# baseline (speedup 1.0000x reference)
"""GNN message passing + global softmax attention + MLP head on 8 TRN2 NeuronCores.

Strategy (node-sharded SPMD, rank enters only via per-core input data):
  - 2 GraphConv layers: aggregation as block-dense adjacency matmul
    aggT[d, dst] = sum_s x_s^T @ A_s with per-core dense count matrix A
    (fp8, exact small ints) kept SBUF-resident across both layers.
  - AllGather h (tiled layout) between layers; AllGather neT before attention.
  - Attention: natural-layout scores S[q, k] on PE, per-q shift from a
    stride-16 subsample max (+margin), exp on ACT with per-partition bias and
    free accum_out for the softmax denominator, xbar DMA-transpose of the
    exp'd tiles, PV matmul accumulating WT[d, q] in PSUM. Mean over q folds
    into a per-q 1/Z scale, one AllReduce of a [128,1] vector.
  - Tiny MLP head replicated on every core (fp32).
"""

import math
import os

import numpy as np
import ml_dtypes

import concourse.bass as bass
import concourse.bacc as bacc
import concourse.tile as tile
from concourse import mybir
from concourse.bass_utils import run_bass_kernel_spmd
from concourse.masks import make_identity

NCORES = 8
NREAL = 10000
NP = 10240           # padded node count
ND = NP // NCORES    # 1280 nodes per core
NT = ND // 128       # 10 q/dst tiles per core
SB = NP // 128       # 80 src blocks
D = 128
KB = 1024            # k block width in attention
KKN = NP // KB       # 10 k blocks
INV = 1.0 / math.sqrt(128.0)
MARGIN = 40.0        # safety margin (scaled units) on the subsample max
KREAL_LAST = NREAL - (KKN - 1) * KB  # 784 valid cols in last k block

BF16 = mybir.dt.bfloat16
FP8 = mybir.dt.float8e4
F32 = mybir.dt.float32

NP_BF16 = mybir.dt.np(BF16)
NP_FP8 = mybir.dt.np(FP8)

_NC_CACHE = {}

RELU = mybir.ActivationFunctionType.Relu
IDENT = mybir.ActivationFunctionType.Identity
EXP = mybir.ActivationFunctionType.Exp
ADD = mybir.AluOpType.add
MULT = mybir.AluOpType.mult
SL3 = ((0, 512), (512, 1024), (1024, 1280))


def _build(phase=9):
    if phase in (51, 52, 53):
        phase_sub, phase = phase, 5
    elif phase in (71, 72, 73):
        phase_sub, phase = phase, 7
    else:
        phase_sub = None
    nc = bacc.Bacc("TRN2", target_bir_lowering=False, debug=False, num_devices=NCORES)

    A_in = nc.dram_tensor("a_cnt", [128, SB, ND], FP8, kind="ExternalInput")
    xt_in = nc.dram_tensor("x_tiled", [128, SB, D], BF16, kind="ExternalInput")
    xTm_in = nc.dram_tensor("xT_mine", [128, ND], BF16, kind="ExternalInput")
    qmask_in = nc.dram_tensor("qmask", [128, NT], F32, kind="ExternalInput")
    names_bf = ["w1r", "w1l", "w2r", "w2l", "wq", "wk", "wv"]
    ins_bf = {n: nc.dram_tensor(n, [D, D], BF16, kind="ExternalInput") for n in names_bf}
    ins_f32 = {
        "b1": nc.dram_tensor("b1", [D, 1], F32, kind="ExternalInput"),
        "b2": nc.dram_tensor("b2", [D, 1], F32, kind="ExternalInput"),
        "qgv": nc.dram_tensor("qgv", [D, 1], F32, kind="ExternalInput"),
        "vgv": nc.dram_tensor("vgv", [D, 1], F32, kind="ExternalInput"),
        "wo": nc.dram_tensor("wo", [D, D], F32, kind="ExternalInput"),
        "wf1": nc.dram_tensor("wf1", [D, 64], F32, kind="ExternalInput"),
        "wf2": nc.dram_tensor("wf2", [64, 32], F32, kind="ExternalInput"),
        "wf3": nc.dram_tensor("wf3", [32, D], F32, kind="ExternalInput"),
        "bo": nc.dram_tensor("bo", [D, 1], F32, kind="ExternalInput"),
        "bf1": nc.dram_tensor("bf1", [64, 1], F32, kind="ExternalInput"),
        "bf2": nc.dram_tensor("bf2", [32, 1], F32, kind="ExternalInput"),
        "bf3": nc.dram_tensor("bf3", [D, 1], F32, kind="ExternalInput"),
    }
    out_t = nc.dram_tensor("out", [1, D], F32, kind="ExternalOutput")
    rg = [list(range(NCORES))]

    with tile.TileContext(nc) as tc:
        with (
            tc.tile_pool(name="dram", bufs=1, space="DRAM") as dram,
            tc.tile_pool(name="const", bufs=1) as cp,
            tc.tile_pool(name="live", bufs=1) as lp,
        ):
            hb_a = dram.tile([128, ND // 2], BF16)
            hb_b = dram.tile([128, ND // 2], BF16)
            hfull_a = dram.tile([NCORES, 128, NT // 2, D], BF16, addr_space="Shared")
            hfull_b = dram.tile([NCORES, 128, NT // 2, D], BF16, addr_space="Shared")
            neb_a = dram.tile([128, ND // 2], BF16)
            neb_b = dram.tile([128, ND // 2], BF16)
            nefull_a = dram.tile([NCORES, 128, ND // 2], BF16, addr_space="Shared")
            nefull_b = dram.tile([NCORES, 128, ND // 2], BF16, addr_space="Shared")
            accb = dram.tile([128, 1], F32)
            accr = dram.tile([128, 1], F32, addr_space="Shared")

            def cload(dram_t, shape, dtype):
                t = cp.tile(shape, dtype, tag=f"c_{dram_t.name}")
                nc.sync.dma_start(out=t[:], in_=dram_t[:])
                return t

            w1r = cload(ins_bf["w1r"], [D, D], BF16)
            w1l = cload(ins_bf["w1l"], [D, D], BF16)
            w2r = cload(ins_bf["w2r"], [D, D], BF16)
            w2l = cload(ins_bf["w2l"], [D, D], BF16)
            wq = cload(ins_bf["wq"], [D, D], BF16)
            wk = cload(ins_bf["wk"], [D, D], BF16)
            wv = cload(ins_bf["wv"], [D, D], BF16)
            b1 = cload(ins_f32["b1"], [D, 1], F32)
            b2 = cload(ins_f32["b2"], [D, 1], F32)
            qgv = cload(ins_f32["qgv"], [D, 1], F32)
            vgv = cload(ins_f32["vgv"], [D, 1], F32)
            wo = cload(ins_f32["wo"], [D, D], F32)
            wf1 = cload(ins_f32["wf1"], [D, 64], F32)
            wf2 = cload(ins_f32["wf2"], [64, 32], F32)
            wf3 = cload(ins_f32["wf3"], [32, D], F32)
            bo = cload(ins_f32["bo"], [D, 1], F32)
            bf1 = cload(ins_f32["bf1"], [64, 1], F32)
            bf2 = cload(ins_f32["bf2"], [32, 1], F32)
            bf3 = cload(ins_f32["bf3"], [D, 1], F32)
            xTm = cload(xTm_in, [128, ND], BF16)
            qmask = cload(qmask_in, [128, NT], F32)
            ident = cp.tile([128, 128], F32)
            make_identity(nc, ident[:])
            ones1 = cp.tile([1, 128], F32)
            nc.gpsimd.memset(ones1[:], 1.0)
            nmarg = cp.tile([128, 1], F32)
            nc.gpsimd.memset(nmarg[:], -MARGIN)

            hT = lp.tile([128, ND], BF16)
            QT = lp.tile([128, ND], BF16)
            neT = lp.tile([128, ND], BF16)

            # ================= message passing =================
            with (
                tc.tile_pool(name="sbL", bufs=1) as sl,
                tc.tile_pool(name="hstp", bufs=4) as hstp,
                tc.tile_pool(name="psL", bufs=1, space="PSUM") as psl,
            ):
                xt = sl.tile([128, SB, D], BF16)
                for j in range(4):
                    nc.sync.dma_start(out=xt[:, 20 * j : 20 * (j + 1), :],
                                      in_=xt_in[:, 20 * j : 20 * (j + 1), :])
                asb = sl.tile([128, SB, ND], FP8)
                for j in range(16):
                    nc.sync.dma_start(
                        out=asb[:, 5 * j : 5 * (j + 1), :],
                        in_=A_in[:, 5 * j : 5 * (j + 1), :],
                    )

                # ----- layer 1 -----
                agg_ps = psl.tile([128, ND], F32, tag="agg", space="PSUM")
                for s in range(SB):
                    for lo, hi in SL3:
                        nc.tensor.matmul(
                            out=agg_ps[:, lo:hi], lhsT=xt[:, s, :],
                            rhs=asb[:, s, lo:hi],
                            start=(s == 0), stop=(s == SB - 1),
                        )
                agg1 = sl.tile([128, ND], BF16, tag="agg1")
                for lo, hi in SL3:
                    nc.vector.tensor_copy(out=agg1[:, lo:hi], in_=agg_ps[:, lo:hi])

                z_ps = psl.tile([128, ND], F32, tag="z", space="PSUM")
                for lo, hi in SL3:
                    nc.tensor.matmul(out=z_ps[:, lo:hi], lhsT=w1r[:],
                                     rhs=xTm[:, lo:hi], start=True, stop=False)
                    nc.tensor.matmul(out=z_ps[:, lo:hi], lhsT=w1l[:],
                                     rhs=agg1[:, lo:hi], start=False, stop=True)
                    nc.scalar.activation(hT[:, lo:hi], z_ps[:, lo:hi], RELU, bias=b1[:])

                if phase >= 2:
                    htiled = sl.tile([128, NT, D], BF16, tag="htiled")
                    nc.sync.dma_start_transpose(out=htiled[:], in_=hT[:])
                    nc.sync.dma_start(
                        out=hb_a[:],
                        in_=htiled[:, 0 : NT // 2, :].rearrange("p t d -> p (t d)"),
                    )
                    nc.sync.dma_start(
                        out=hb_b[:],
                        in_=htiled[:, NT // 2 :, :].rearrange("p t d -> p (t d)"),
                    )
                if phase >= 3:
                    nc.gpsimd.collective_compute(
                        "AllGather", mybir.AluOpType.bypass, replica_groups=rg,
                        ins=[hb_a[:].opt()], outs=[hfull_a[:].opt()],
                    )
                    nc.gpsimd.collective_compute(
                        "AllGather", mybir.AluOpType.bypass, replica_groups=rg,
                        ins=[hb_b[:].opt()], outs=[hfull_b[:].opt()],
                    )

                # ----- layer 2 -----
                if phase >= 4:
                    agg_ps2 = psl.tile([128, ND], F32, tag="agg", space="PSUM")
                    hts_a = sl.tile([128, NCORES, NT // 2, D], BF16, tag="hts_a")
                    hts_b = sl.tile([128, NCORES, NT // 2, D], BF16, tag="hts_b")
                    for c in range(NCORES):
                        nc.sync.dma_start(out=hts_a[:, c, :, :], in_=hfull_a[c])
                    for c in range(NCORES):
                        nc.sync.dma_start(out=hts_b[:, c, :, :], in_=hfull_b[c])
                    sorder = [c * NT + t for t in range(NT) for c in range(NCORES)]
                    for si, s in enumerate(sorder):
                        c, t_loc = s // NT, s % NT
                        hsl = (hts_a if t_loc < NT // 2 else hts_b)[
                            :, c, t_loc % (NT // 2), :
                        ]
                        for lo, hi in SL3:
                            nc.tensor.matmul(
                                out=agg_ps2[:, lo:hi], lhsT=hsl,
                                rhs=asb[:, s, lo:hi],
                                start=(si == 0), stop=(si == SB - 1),
                            )
                    agg2 = sl.tile([128, ND], BF16, tag="agg2")
                    for lo, hi in SL3:
                        nc.vector.tensor_copy(out=agg2[:, lo:hi], in_=agg_ps2[:, lo:hi])

                    z_ps2 = psl.tile([128, ND], F32, tag="z", space="PSUM")
                    for lo, hi in SL3:
                        nc.tensor.matmul(out=z_ps2[:, lo:hi], lhsT=w2r[:],
                                         rhs=hT[:, lo:hi], start=True, stop=False)
                        nc.tensor.matmul(out=z_ps2[:, lo:hi], lhsT=w2l[:],
                                         rhs=agg2[:, lo:hi], start=False, stop=True)
                        nc.scalar.activation(neT[:, lo:hi], z_ps2[:, lo:hi], IDENT,
                                             bias=b2[:])

                    q_ps = psl.tile([128, ND], F32, tag="agg", space="PSUM")
                    for lo, hi in SL3:
                        nc.tensor.matmul(out=q_ps[:, lo:hi], lhsT=wq[:],
                                         rhs=neT[:, lo:hi], start=True, stop=True)
                        nc.vector.tensor_scalar(
                            out=QT[:, lo:hi], in0=q_ps[:, lo:hi],
                            scalar1=qgv[:], scalar2=None, op0=ADD,
                        )

                    nc.sync.dma_start(out=neb_a[:], in_=neT[:, 0 : ND // 2])
                    nc.gpsimd.collective_compute(
                        "AllGather", mybir.AluOpType.bypass, replica_groups=rg,
                        ins=[neb_a[:].opt()], outs=[nefull_a[:].opt()],
                    )
                    nc.sync.dma_start(out=neb_b[:], in_=neT[:, ND // 2 :])
                    nc.gpsimd.collective_compute(
                        "AllGather", mybir.AluOpType.bypass, replica_groups=rg,
                        ins=[neb_b[:].opt()], outs=[nefull_b[:].opt()],
                    )

            # ================= attention =================
            if phase >= 5:
                with (
                    tc.tile_pool(name="sbA", bufs=1) as sa,
                    tc.tile_pool(name="etp", bufs=2) as etp,
                    tc.tile_pool(name="ep", bufs=8) as ep,
                    tc.tile_pool(name="stp", bufs=2, space="PSUM") as stp,
                    tc.tile_pool(name="wtp", bufs=1, space="PSUM") as wtp,
                ):
                    neTf = sa.tile([128, NCORES, ND], BF16)
                    for r in range(NCORES):
                        nc.sync.dma_start(out=neTf[:, r, 0 : ND // 2],
                                          in_=nefull_a[r])
                    for r in range(NCORES):
                        nc.sync.dma_start(out=neTf[:, r, ND // 2 :],
                                          in_=nefull_b[r])
                    neTfl = neTf[:].rearrange("p r j -> p (r j)")

                    KT = sa.tile([128, NP], BF16)
                    for i in range(NP // 512):
                        kps = stp.tile([128, KB], F32, tag="st", space="PSUM")
                        nc.tensor.matmul(out=kps[:, 0:512], lhsT=wk[:],
                                         rhs=neTfl[:, 512 * i : 512 * (i + 1)],
                                         start=True, stop=True)
                        nc.vector.tensor_copy(out=KT[:, 512 * i : 512 * (i + 1)],
                                              in_=kps[:, 0:512])

                    vt = sa.tile([128, SB, D], BF16)
                    for g in range(SB // 8 if phase_sub != 51 else 0):
                        vps = stp.tile([128, KB], F32, tag="st", space="PSUM")
                        for j in range(8):
                            kc = 8 * g + j
                            nc.tensor.matmul(
                                out=vps[:, 128 * j : 128 * (j + 1)],
                                lhsT=neTfl[:, 128 * kc : 128 * (kc + 1)],
                                rhs=wv[:], start=True, stop=True,
                            )
                        nc.vector.tensor_copy(
                            out=vt[:, 8 * g : 8 * (g + 1), :].rearrange(
                                "p s d -> p (s d)"
                            ),
                            in_=vps[:],
                        )

                    KTs = sa.tile([128, 640], BF16)
                    if phase_sub not in (51, 52):
                        nc.vector.tensor_copy(out=KTs[:], in_=KT[:, ::16])
                    negc = sa.tile([128, NT], F32)
                    for t in range(NT if phase_sub not in (51, 52) else 0):
                        sps = stp.tile([128, KB], F32, tag="st", space="PSUM")
                        nc.tensor.matmul(out=sps[:, 0:512],
                                         lhsT=QT[:, 128 * t : 128 * (t + 1)],
                                         rhs=KTs[:, 0:512], start=True, stop=True)
                        nc.tensor.matmul(out=sps[:, 512:640],
                                         lhsT=QT[:, 128 * t : 128 * (t + 1)],
                                         rhs=KTs[:, 512:640], start=True, stop=True)
                        mh = sa.tile([128, 1], F32, tag="mh")
                        nc.vector.reduce_max(mh[:], sps[:, 0:640],
                                             axis=mybir.AxisListType.X)
                        nc.scalar.activation(negc[:, t : t + 1], mh[:], IDENT,
                                             scale=-INV, bias=nmarg[:])

                    WT = wtp.tile([128, ND], F32, tag="wt", space="PSUM")
                    Zbuf = sa.tile([128, NT, KKN], F32)
                    if phase >= 6:
                        def pv_mms(kk, ET2kk):
                            for j in range(KB // 128):
                                kc = (KB // 128) * kk + j
                                for gi, (t0, t1) in enumerate(((0, 4), (4, 8), (8, 10))):
                                    yield (kk, kc, j, gi, t0, t1, ET2kk)

                        def emit_pv_mm(mm):
                            kk, kc, j, gi, t0, t1, ET2kk = mm
                            nc.tensor.matmul(
                                out=WT[:, 128 * t0 : 128 * t1],
                                lhsT=vt[:, kc, :],
                                rhs=ET2kk[:, t0:t1, j, :],
                                start=(kk == 0 and j == 0),
                                stop=(kk == KKN - 1 and j == KB // 128 - 1),
                            )

                        pending_pv = None
                        for kk in range(KKN):
                            ET2 = etp.tile([128, NT, KB // 128, 128], BF16, tag="et2")
                            for t in range(NT):
                                stt = stp.tile([128, KB], F32, tag="st", space="PSUM")
                                for lo, hi in ((0, 512), (512, 1024)):
                                    nc.tensor.matmul(
                                        out=stt[:, lo:hi],
                                        lhsT=QT[:, 128 * t : 128 * (t + 1)],
                                        rhs=KT[:, KB * kk + lo : KB * kk + hi],
                                        start=True, stop=True,
                                    )
                                Et = ep.tile([128, KB], BF16, tag="e")
                                nc.scalar.activation(
                                    Et[:], stt[:], EXP, scale=INV,
                                    bias=negc[:, t : t + 1],
                                )
                                if kk == KKN - 1:
                                    nc.gpsimd.memset(Et[:, KREAL_LAST:], 0)
                                nc.vector.reduce_sum(
                                    Zbuf[:, t, kk : kk + 1],
                                    Et[:, 0 : (KB if kk < KKN - 1 else KREAL_LAST)],
                                    axis=mybir.AxisListType.X,
                                )
                                nc.sync.dma_start_transpose(
                                    out=ET2[:, t, :, :], in_=Et[:]
                                )
                                if pending_pv is not None:
                                    lo2 = (24 * t) // NT
                                    hi2 = (24 * (t + 1)) // NT
                                    for mm in pending_pv[lo2:hi2]:
                                        emit_pv_mm(mm)
                            pending_pv = list(pv_mms(kk, ET2))
                        for mm in pending_pv:
                            emit_pv_mm(mm)

                    if phase >= 7:
                        acc_prev = None
                        junk = sa.tile([128, 128], F32, tag="junk")
                        accs = sa.tile([128, NT], F32)
                        for t in range(NT):
                            zt = sa.tile([128, 1], F32, tag="zt")
                            nc.vector.reduce_sum(zt[:], Zbuf[:, t, :],
                                                 axis=mybir.AxisListType.X)
                            rz = sa.tile([128, 1], F32, tag="rz")
                            nc.vector.reciprocal(rz[:], zt[:])
                            rzm = sa.tile([128, 1], F32, tag="rzm")
                            nc.vector.tensor_tensor(out=rzm[:], in0=rz[:],
                                                    in1=qmask[:, t : t + 1], op=MULT)
                            tp = stp.tile([128, KB], F32, tag="st", space="PSUM")
                            nc.tensor.transpose(out=tp[:1, 0:128], in_=rzm[:],
                                                identity=ident[:])
                            rzrow = sa.tile([1, 128], F32, tag="rzrow")
                            nc.vector.tensor_copy(out=rzrow[:], in_=tp[:1, 0:128])
                            rzb_ps = stp.tile([128, KB], F32, tag="st", space="PSUM")
                            nc.tensor.matmul(out=rzb_ps[:, 0:128], lhsT=ones1[:],
                                             rhs=rzrow[:], start=True, stop=True)
                            rzb = sa.tile([128, 128], F32, tag="rzb")
                            nc.vector.tensor_copy(out=rzb[:], in_=rzb_ps[:, 0:128])
                            nc.vector.tensor_tensor(
                                out=junk[:], in0=WT[:, 128 * t : 128 * (t + 1)],
                                in1=rzb[:], op=MULT,
                            )
                            nc.vector.reduce_sum(accs[:, t : t + 1], junk[:],
                                                 axis=mybir.AxisListType.X)

                        if phase_sub == 71:
                            nc.gpsimd.dma_start(out=out_t[:, 0:10], in_=accs[:1, :])
                        accsb = sa.tile([128, 1], F32, tag="accsb")
                        nc.vector.reduce_sum(accsb[:], accs[:],
                                             axis=mybir.AxisListType.X)
                        nc.sync.dma_start(out=accb[:], in_=accsb[:])
                        nc.gpsimd.collective_compute(
                            "AllReduce", ADD, replica_groups=rg,
                            ins=[accb[:].opt()], outs=[accr[:].opt()],
                        )
                        if phase_sub != 71:
                            if phase_sub == 72:
                                nc.gpsimd.dma_start(out=out_t[:, 0:1], in_=accsb[:1, :])
                            if phase_sub != 72:
                                accg = sa.tile([128, 1], F32, tag="accg")
                                nc.sync.dma_start(out=accg[:], in_=accr[:])
                                aggc = sa.tile([128, 1], F32, tag="aggc")
                                nc.scalar.activation(aggc[:], accg[:], IDENT,
                                                     scale=1.0 / NREAL, bias=vgv[:])

                                hps = stp.tile([128, KB], F32, tag="st", space="PSUM")
                                nc.tensor.matmul(out=hps[:, 0:1], lhsT=wo[:], rhs=aggc[:],
                                                 start=True, stop=True)
                                state = sa.tile([128, 1], F32, tag="state")
                                nc.scalar.activation(state[:], hps[:, 0:1], IDENT, bias=bo[:])
                                hps2 = stp.tile([128, KB], F32, tag="st", space="PSUM")
                                nc.tensor.matmul(out=hps2[:64, 0:1], lhsT=wf1[:], rhs=state[:],
                                                 start=True, stop=True)
                                x1 = sa.tile([64, 1], F32, tag="x1")
                                nc.scalar.activation(x1[:], hps2[:64, 0:1], RELU, bias=bf1[:])
                                hps3 = stp.tile([128, KB], F32, tag="st", space="PSUM")
                                nc.tensor.matmul(out=hps3[:32, 0:1], lhsT=wf2[:], rhs=x1[:],
                                                 start=True, stop=True)
                                x2 = sa.tile([32, 1], F32, tag="x2")
                                nc.scalar.activation(x2[:], hps3[:32, 0:1], RELU, bias=bf2[:])
                                hps4 = stp.tile([128, KB], F32, tag="st", space="PSUM")
                                nc.tensor.matmul(out=hps4[:, 0:1], lhsT=wf3[:], rhs=x2[:],
                                                 start=True, stop=True)
                                lg = sa.tile([128, 1], F32, tag="lg")
                                nc.scalar.activation(lg[:], hps4[:, 0:1], IDENT, bias=bf3[:])
                                if phase_sub == 73:
                                    nc.gpsimd.dma_start(out=out_t[:, 0:1], in_=lg[:1, :])
                                if phase_sub != 73:
                                    tps = stp.tile([128, KB], F32, tag="st", space="PSUM")
                                    nc.tensor.transpose(out=tps[:1, 0:128], in_=lg[:], identity=ident[:])
                                    er = sa.tile([1, 128], F32, tag="er")
                                    zf = sa.tile([1, 1], F32, tag="zf")
                                    nc.scalar.activation(er[:], tps[:1, 0:128], EXP, accum_out=zf[:])
                                    rzf = sa.tile([1, 1], F32, tag="rzf")
                                    nc.vector.reciprocal(rzf[:], zf[:])
                                    orow = sa.tile([1, 128], F32, tag="orow")
                                    nc.vector.tensor_scalar(out=orow[:], in0=er[:], scalar1=rzf[:],
                                                            scalar2=None, op0=MULT)
                                    nc.sync.dma_start(out=out_t[:], in_=orow[:])



                    elif phase == 6:
                        nc.gpsimd.dma_start(out=out_t[:, 0:100], in_=Zbuf[:1, :, :].rearrange("p t k -> p (t k)"))
                    else:
                        nc.gpsimd.dma_start(out=out_t[:], in_=KT[:1, 0:128])
            else:
                src = {1: hT, 2: hT, 3: hT, 4: neT}[phase]
                nc.gpsimd.dma_start(out=out_t[:], in_=src[:1, 0:128])

    nc.compile()
    return nc


def _get_nc():
    phase = int(os.environ.get("K_PHASE", "9"))
    key = ("nc", phase)
    if key not in _NC_CACHE:
        _NC_CACHE[key] = _build(phase)
    return _NC_CACHE[key]


def _prep_in_maps(inputs):
    f32 = np.float32
    x = np.asarray(inputs["node_features"], f32)
    g = np.asarray(inputs["global_info"], f32)
    ei = np.asarray(inputs["edge_index"])
    src = np.asarray(ei[0], np.int64)
    dst = np.asarray(ei[1], np.int64)

    xp = np.zeros((NP, D), f32)
    xp[:NREAL] = x
    xb = xp.astype(NP_BF16)
    x_tiled = np.ascontiguousarray(xb.reshape(SB, 128, D).transpose(1, 0, 2))

    qgv = (np.asarray(inputs["bQ"], f32)
           + (g @ np.asarray(inputs["WQg"], f32))[0]
           + np.asarray(inputs["bQg"], f32)).reshape(D, 1)
    vgv = (np.asarray(inputs["bV"], f32)
           + (g @ np.asarray(inputs["WVg"], f32))[0]
           + np.asarray(inputs["bVg"], f32)).reshape(D, 1)

    def bf(name):
        return np.ascontiguousarray(np.asarray(inputs[name], f32).astype(NP_BF16))

    shared = {
        "w1r": bf("W1_root"), "w1l": bf("W1_rel"),
        "w2r": bf("W2_root"), "w2l": bf("W2_rel"),
        "wq": bf("WQ"), "wk": bf("WK"), "wv": bf("WV"),
        "b1": np.asarray(inputs["b1"], f32).reshape(D, 1),
        "b2": np.asarray(inputs["b2"], f32).reshape(D, 1),
        "qgv": qgv, "vgv": vgv,
        "wo": np.asarray(inputs["Wo"], f32),
        "wf1": np.asarray(inputs["Wfc1"], f32),
        "wf2": np.asarray(inputs["Wfc2"], f32),
        "wf3": np.asarray(inputs["Wfc3"], f32),
        "bo": np.asarray(inputs["bo"], f32).reshape(D, 1),
        "bf1": np.asarray(inputs["bfc1"], f32).reshape(64, 1),
        "bf2": np.asarray(inputs["bfc2"], f32).reshape(32, 1),
        "bf3": np.asarray(inputs["bfc3"], f32).reshape(D, 1),
        "x_tiled": x_tiled,
    }

    core_of = dst // ND
    in_maps = []
    nodes = np.arange(NP)
    for c in range(NCORES):
        m = core_of == c
        A = np.zeros((NP, ND), f32)
        np.add.at(A, (src[m], dst[m] - ND * c), 1.0)
        Ac = np.ascontiguousarray(
            A.reshape(SB, 128, ND).transpose(1, 0, 2)
        ).astype(NP_FP8)
        xTm = np.ascontiguousarray(xb[ND * c : ND * (c + 1)].T)
        qm = (nodes[ND * c : ND * (c + 1)] < NREAL).astype(f32)
        qmask = np.ascontiguousarray(qm.reshape(NT, 128).T)
        in_maps.append({**shared, "a_cnt": Ac, "xT_mine": xTm, "qmask": qmask})
    return in_maps


def kernel(**inputs):
    nc = _get_nc()
    in_maps = _prep_in_maps(inputs)
    res = run_bass_kernel_spmd(nc, in_maps, core_ids=list(range(NCORES)))
    return np.asarray(res.results[0]["out"], np.float32)



# revision 6
# speedup vs baseline: 1.1994x; 1.1994x over previous
"""GNN message passing + global softmax attention + MLP head on 8 TRN2 NeuronCores.

v2 strategy (node/dst-sharded SPMD, single program, rank enters via data):
  - GraphConv aggregation as block-dense adjacency matmuls (fp8 count matrix,
    SBUF-resident across both layers), same as v1.
  - Transposed-score attention: S^T[k, q] = K'^T Q is computed with k on
    partitions so the exp'd tiles feed the PV matmul directly (lhsT = V tile,
    rhs = exp tile) -- no DMA transpose of the score tiles at all.
  - Numerical range without a per-row max pass: host fits a linear model
    c_q ~= Q_q . u + beta to the (subsampled) row max of the scores, then
    folds -sqrt(128)*u into the K bias (per-q shift is constant along k, so
    softmax is mathematically unchanged) and -beta into the exp bias.
  - Softmax denominator accumulated on the Vector engine (Zacc += Et per
    k-tile), partition-reduced once at the end with a ones-matmul.
  - Mean over q: W[d, q] stays PSUM-resident (3 banks), divided by Z via a
    broadcast row, one AllReduce of [128, 1], replicated MLP head.
  - Collectives: h (between layers) in 2 chunks, K^T/V (before attention) in
    4 chunks, all overlapped with compute; k-tiles consumed in arrival order.
"""

import math
import os

import numpy as np
import ml_dtypes
import scipy.sparse as sp

import concourse.bass as bass
import concourse.bacc as bacc
import concourse.tile as tile
from concourse import mybir
from concourse.bass_utils import run_bass_kernel_spmd
from concourse.masks import make_identity

NCORES = 8
NREAL = 10000
NP = 10240           # padded node count
ND = NP // NCORES    # 1280 nodes per core
NT = ND // 128       # 10 tiles of 128 per core
SB = NP // 128       # 80 src blocks globally
D = 128
INV = 1.0 / math.sqrt(128.0)

BF16 = mybir.dt.bfloat16
FP8 = mybir.dt.float8e4
F32 = mybir.dt.float32

NP_BF16 = mybir.dt.np(BF16)
NP_FP8 = mybir.dt.np(FP8)

_NC_CACHE = {}

RELU = mybir.ActivationFunctionType.Relu
IDENT = mybir.ActivationFunctionType.Identity
EXP = mybir.ActivationFunctionType.Exp
ADD = mybir.AluOpType.add
MULT = mybir.AluOpType.mult
SL3 = ((0, 512), (512, 1024), (1024, 1280))
QA = ((0, 512), (512, 1024))   # score q-chunk A (1024 wide)
QB = (1024, 1280)              # score q-chunk B (256 wide)


def _build(phase=9):
    nc = bacc.Bacc("TRN2", target_bir_lowering=False, debug=False, num_devices=NCORES)

    A_in = nc.dram_tensor("a_cnt", [128, SB, ND], FP8, kind="ExternalInput")
    xt_in = nc.dram_tensor("x_tiled", [128, SB, D], BF16, kind="ExternalInput")
    xTm_in = nc.dram_tensor("xT_mine", [128, ND], BF16, kind="ExternalInput")
    qmr_in = nc.dram_tensor("qmask_row", [1, ND], F32, kind="ExternalInput")
    names_bf = ["w1r", "w1l", "w2r", "w2l", "wq", "wk", "wv"]
    ins_bf = {n: nc.dram_tensor(n, [D, D], BF16, kind="ExternalInput") for n in names_bf}
    ins_f32 = {
        "b1": nc.dram_tensor("b1", [D, 1], F32, kind="ExternalInput"),
        "b2": nc.dram_tensor("b2", [D, 1], F32, kind="ExternalInput"),
        "qgv": nc.dram_tensor("qgv", [D, 1], F32, kind="ExternalInput"),
        "vgv": nc.dram_tensor("vgv", [D, 1], F32, kind="ExternalInput"),
        "negv": nc.dram_tensor("negv", [D, 1], F32, kind="ExternalInput"),
        "negbeta": nc.dram_tensor("negbeta", [D, 1], F32, kind="ExternalInput"),
        "wo": nc.dram_tensor("wo", [D, D], F32, kind="ExternalInput"),
        "wf1": nc.dram_tensor("wf1", [D, 64], F32, kind="ExternalInput"),
        "wf2": nc.dram_tensor("wf2", [64, 32], F32, kind="ExternalInput"),
        "wf3": nc.dram_tensor("wf3", [32, D], F32, kind="ExternalInput"),
        "bo": nc.dram_tensor("bo", [D, 1], F32, kind="ExternalInput"),
        "bf1": nc.dram_tensor("bf1", [64, 1], F32, kind="ExternalInput"),
        "bf2": nc.dram_tensor("bf2", [32, 1], F32, kind="ExternalInput"),
        "bf3": nc.dram_tensor("bf3", [D, 1], F32, kind="ExternalInput"),
    }
    out_t = nc.dram_tensor("out", [1, D], F32, kind="ExternalOutput")
    rg = [list(range(NCORES))]

    with tile.TileContext(nc) as tc:
        with (
            tc.tile_pool(name="dram", bufs=1, space="DRAM") as dram,
            tc.tile_pool(name="const", bufs=1) as cp,
            tc.tile_pool(name="live", bufs=1) as lp,
        ):
            hb_a = dram.tile([128, 640], BF16)
            hb_b = dram.tile([128, 640], BF16)
            hfull_a = dram.tile([NCORES, 128, 640], BF16, addr_space="Shared")
            hfull_b = dram.tile([NCORES, 128, 640], BF16, addr_space="Shared")
            kva1 = dram.tile([128, 640], BF16)
            kva2 = dram.tile([128, 640], BF16)
            kvb1 = dram.tile([128, 640], BF16)
            kvb2 = dram.tile([128, 640], BF16)
            kvaf1 = dram.tile([NCORES, 128, 640], BF16, addr_space="Shared")
            kvaf2 = dram.tile([NCORES, 128, 640], BF16, addr_space="Shared")
            kvbf1 = dram.tile([NCORES, 128, 640], BF16, addr_space="Shared")
            kvbf2 = dram.tile([NCORES, 128, 640], BF16, addr_space="Shared")
            accb = dram.tile([128, 1], F32)
            accr = dram.tile([128, 1], F32, addr_space="Shared")

            def cload(dram_t, shape, dtype):
                t = cp.tile(shape, dtype, tag=f"c_{dram_t.name}")
                nc.sync.dma_start(out=t[:], in_=dram_t[:])
                return t

            w1r = cload(ins_bf["w1r"], [D, D], BF16)
            w1l = cload(ins_bf["w1l"], [D, D], BF16)
            w2r = cload(ins_bf["w2r"], [D, D], BF16)
            w2l = cload(ins_bf["w2l"], [D, D], BF16)
            wq = cload(ins_bf["wq"], [D, D], BF16)
            wk = cload(ins_bf["wk"], [D, D], BF16)
            wv = cload(ins_bf["wv"], [D, D], BF16)
            b1 = cload(ins_f32["b1"], [D, 1], F32)
            b2 = cload(ins_f32["b2"], [D, 1], F32)
            qgv = cload(ins_f32["qgv"], [D, 1], F32)
            vgv = cload(ins_f32["vgv"], [D, 1], F32)
            negv = cload(ins_f32["negv"], [D, 1], F32)
            negbeta = cload(ins_f32["negbeta"], [D, 1], F32)
            wo = cload(ins_f32["wo"], [D, D], F32)
            wf1 = cload(ins_f32["wf1"], [D, 64], F32)
            wf2 = cload(ins_f32["wf2"], [64, 32], F32)
            wf3 = cload(ins_f32["wf3"], [32, D], F32)
            bo = cload(ins_f32["bo"], [D, 1], F32)
            bf1 = cload(ins_f32["bf1"], [64, 1], F32)
            bf2 = cload(ins_f32["bf2"], [32, 1], F32)
            bf3 = cload(ins_f32["bf3"], [D, 1], F32)
            xTm = cload(xTm_in, [128, ND], BF16)
            qmrow = cload(qmr_in, [1, ND], F32)
            ident = cp.tile([128, 128], F32)
            make_identity(nc, ident[:])
            ones1 = cp.tile([1, 128], F32)
            nc.gpsimd.memset(ones1[:], 1.0)
            ones128 = cp.tile([128, 1], F32)
            nc.gpsimd.memset(ones128[:], 1.0)

            hT = lp.tile([128, ND], BF16)
            neT = lp.tile([128, ND], BF16)
            htiled = lp.tile([128, NT, D], BF16)

            # ================= message passing =================
            with (
                tc.tile_pool(name="sbL", bufs=1) as sl,
                tc.tile_pool(name="psL", bufs=1, space="PSUM") as psl,
            ):
                xt = sl.tile([128, SB, D], BF16)
                for j in range(4):
                    nc.sync.dma_start(out=xt[:, 20 * j : 20 * (j + 1), :],
                                      in_=xt_in[:, 20 * j : 20 * (j + 1), :])
                asb = sl.tile([128, SB, ND], FP8)
                for j in range(16):
                    nc.sync.dma_start(
                        out=asb[:, 5 * j : 5 * (j + 1), :],
                        in_=A_in[:, 5 * j : 5 * (j + 1), :],
                    )

                # ----- layer 1 -----
                z1 = psl.tile([128, ND], F32, tag="z", space="PSUM")
                for lo, hi in SL3:
                    nc.tensor.matmul(out=z1[:, lo:hi], lhsT=w1r[:],
                                     rhs=xTm[:, lo:hi], start=True, stop=False)
                agg_ps = psl.tile([128, ND], F32, tag="agg", space="PSUM")
                for s in range(SB):
                    for lo, hi in SL3:
                        nc.tensor.matmul(
                            out=agg_ps[:, lo:hi], lhsT=xt[:, s, :],
                            rhs=asb[:, s, lo:hi],
                            start=(s == 0), stop=(s == SB - 1),
                        )
                agg1 = sl.tile([128, ND], BF16, tag="agg1")
                for lo, hi in SL3:
                    nc.vector.tensor_copy(out=agg1[:, lo:hi], in_=agg_ps[:, lo:hi])
                for lo, hi in SL3:
                    nc.tensor.matmul(out=z1[:, lo:hi], lhsT=w1l[:],
                                     rhs=agg1[:, lo:hi], start=False, stop=True)
                    nc.scalar.activation(hT[:, lo:hi], z1[:, lo:hi], RELU, bias=b1[:])

                nc.sync.dma_start_transpose(out=htiled[:], in_=hT[:])
                nc.sync.dma_start(
                    out=hb_a[:],
                    in_=htiled[:, 0:5, :].rearrange("p t d -> p (t d)"))
                nc.gpsimd.collective_compute(
                    "AllGather", mybir.AluOpType.bypass, replica_groups=rg,
                    ins=[hb_a[:].opt()], outs=[hfull_a[:].opt()],
                )
                nc.sync.dma_start(
                    out=hb_b[:],
                    in_=htiled[:, 5:10, :].rearrange("p t d -> p (t d)"))
                nc.gpsimd.collective_compute(
                    "AllGather", mybir.AluOpType.bypass, replica_groups=rg,
                    ins=[hb_b[:].opt()], outs=[hfull_b[:].opt()],
                )

                # ----- layer 2 (root term first: it needs no collective) -----
                z2 = psl.tile([128, ND], F32, tag="z", space="PSUM")
                for lo, hi in SL3:
                    nc.tensor.matmul(out=z2[:, lo:hi], lhsT=w2r[:],
                                     rhs=hT[:, lo:hi], start=True, stop=False)

                hts_a = sl.tile([128, NCORES, 5, D], BF16, tag="hts_a")
                for c in range(NCORES):
                    nc.sync.dma_start(out=hts_a[:, c, :, :], in_=hfull_a[c])
                hts_b = sl.tile([128, NCORES, 5, D], BF16, tag="hts_b")
                for c in range(NCORES):
                    nc.sync.dma_start(out=hts_b[:, c, :, :], in_=hfull_b[c])

                agg_ps2 = psl.tile([128, ND], F32, tag="agg", space="PSUM")
                # consume chunk-a blocks first so chunk b overlaps with compute
                sorder = ([(c, t) for c in range(NCORES) for t in range(5)]
                          + [(c, t) for c in range(NCORES) for t in range(5, NT)])
                for si, (c, t) in enumerate(sorder):
                    hsl = (hts_a if t < 5 else hts_b)[:, c, t % 5, :]
                    s = c * NT + t
                    for lo, hi in SL3:
                        nc.tensor.matmul(
                            out=agg_ps2[:, lo:hi], lhsT=hsl,
                            rhs=asb[:, s, lo:hi],
                            start=(si == 0), stop=(si == SB - 1),
                        )
                agg2 = sl.tile([128, ND], BF16, tag="agg2")
                for lo, hi in SL3:
                    nc.vector.tensor_copy(out=agg2[:, lo:hi], in_=agg_ps2[:, lo:hi])
                for lo, hi in SL3:
                    nc.tensor.matmul(out=z2[:, lo:hi], lhsT=w2l[:],
                                     rhs=agg2[:, lo:hi], start=False, stop=True)
                    nc.scalar.activation(neT[:, lo:hi], z2[:, lo:hi], IDENT,
                                         bias=b2[:])

            # ================= attention =================
            with tc.tile_pool(name="sbA", bufs=1) as sa:
                kv = sa.tile([128, 2 * ND], BF16)
                QT = sa.tile([128, ND], BF16)
                Zacc = sa.tile([128, ND], F32)
                KTf = sa.tile([128, NCORES, NT, D], BF16)
                vtf = sa.tile([128, NCORES, NT, D], BF16)

                with tc.tile_pool(name="kvq", bufs=2, space="PSUM") as kvqp:
                    # K'^T = WK^T neT - v  (no bK: constant-in-k shifts cancel;
                    # -v applies the fitted per-q stabilizer via the K side)
                    for i, (lo, hi) in enumerate(SL3):
                        kps = kvqp.tile([128, 512], F32, tag="kvq", space="PSUM")
                        nc.tensor.matmul(out=kps[:, 0 : hi - lo], lhsT=wk[:],
                                         rhs=neT[:, lo:hi], start=True, stop=True)
                        nc.scalar.activation(kv[:, lo:hi], kps[:, 0 : hi - lo],
                                             IDENT, bias=negv[:])
                        if i == 1:
                            nc.sync.dma_start(out=kva1[:], in_=kv[:, 0:640])
                            nc.gpsimd.collective_compute(
                                "AllGather", mybir.AluOpType.bypass,
                                replica_groups=rg,
                                ins=[kva1[:].opt()], outs=[kvaf1[:].opt()],
                            )
                    nc.sync.dma_start(out=kva2[:], in_=kv[:, 640:1280])
                    nc.gpsimd.collective_compute(
                        "AllGather", mybir.AluOpType.bypass, replica_groups=rg,
                        ins=[kva2[:].opt()], outs=[kvaf2[:].opt()],
                    )
                    # V tiled [k, d]: one matmul per 128-block of local nodes
                    for g, nblk in ((0, 4), (1, 4), (2, 2)):
                        vps = kvqp.tile([128, 512], F32, tag="kvq", space="PSUM")
                        for j in range(nblk):
                            b = 4 * g + j
                            nc.tensor.matmul(
                                out=vps[:, 128 * j : 128 * (j + 1)],
                                lhsT=neT[:, 128 * b : 128 * (b + 1)],
                                rhs=wv[:], start=True, stop=True,
                            )
                        nc.vector.tensor_copy(
                            out=kv[:, ND + 512 * g : ND + 512 * g + 128 * nblk],
                            in_=vps[:, 0 : 128 * nblk],
                        )
                        if g == 1:
                            nc.sync.dma_start(out=kvb1[:], in_=kv[:, ND : ND + 640])
                            nc.gpsimd.collective_compute(
                                "AllGather", mybir.AluOpType.bypass,
                                replica_groups=rg,
                                ins=[kvb1[:].opt()], outs=[kvbf1[:].opt()],
                            )
                    nc.sync.dma_start(out=kvb2[:], in_=kv[:, ND + 640 : ND + 1280])
                    nc.gpsimd.collective_compute(
                        "AllGather", mybir.AluOpType.bypass, replica_groups=rg,
                        ins=[kvb2[:].opt()], outs=[kvbf2[:].opt()],
                    )
                    # Q while the gathers fly
                    for lo, hi in SL3:
                        qps = kvqp.tile([128, 512], F32, tag="kvq", space="PSUM")
                        nc.tensor.matmul(out=qps[:, 0 : hi - lo], lhsT=wq[:],
                                         rhs=neT[:, lo:hi], start=True, stop=True)
                        nc.scalar.activation(QT[:, lo:hi], qps[:, 0 : hi - lo],
                                             IDENT, bias=qgv[:])

                for r in range(NCORES):
                    nc.sync.dma_start(out=KTf[:, r, 0:5, :], in_=kvaf1[r])
                for r in range(NCORES):
                    nc.sync.dma_start(out=vtf[:, r, 0:5, :], in_=kvbf1[r])
                for r in range(NCORES):
                    nc.sync.dma_start(out=KTf[:, r, 5:10, :], in_=kvaf2[r])
                for r in range(NCORES):
                    nc.sync.dma_start(out=vtf[:, r, 5:10, :], in_=kvbf2[r])

                # k-tile order: arrival order of the chunked gathers.
                # (7, 9) is all padding -- skipped entirely.
                korder = [(r, t) for g in (0, 1) for r in range(NCORES)
                          for t in range(5 * g, 5 * g + 5) if (r, t) != (7, 9)]
                nlast = len(korder) - 1

                with tc.tile_pool(name="wtp", bufs=1, space="PSUM") as wtp:
                    wta = wtp.tile([128, 1024], F32, tag="wta", space="PSUM")
                    wtb = wtp.tile([128, 256], F32, tag="wtb", space="PSUM")

                    with (
                        tc.tile_pool(name="stp", bufs=2, space="PSUM") as stp,
                        tc.tile_pool(name="ep", bufs=3) as ep,
                    ):
                        pending = None

                        def emit_pv(pv):
                            eta, etb, vt, first, last = pv
                            for lo, hi in QA:
                                nc.tensor.matmul(
                                    out=wta[:, lo:hi], lhsT=vt,
                                    rhs=eta[:, lo:hi],
                                    start=first, stop=last,
                                )
                            nc.tensor.matmul(
                                out=wtb[:], lhsT=vt, rhs=etb[:],
                                start=first, stop=last,
                            )

                        for si, (r, t) in enumerate(korder):
                            kt = KTf[:, r, t, :]
                            sta = stp.tile([128, 1024], F32, tag="st", space="PSUM")
                            for lo, hi in QA:
                                nc.tensor.matmul(out=sta[:, lo:hi], lhsT=kt,
                                                 rhs=QT[:, lo:hi],
                                                 start=True, stop=True)
                            stb = stp.tile([128, 1024], F32, tag="st", space="PSUM")
                            nc.tensor.matmul(out=stb[:, 0:256], lhsT=kt,
                                             rhs=QT[:, QB[0]:QB[1]],
                                             start=True, stop=True)
                            eta = ep.tile([128, 1024], BF16, tag="eta")
                            etb = ep.tile([128, 256], BF16, tag="etb")
                            if (r, t) == (7, 8):
                                # global nodes 10000.. are padding: zero the
                                # tile, then exp only the 16 real rows
                                nc.gpsimd.memset(eta[:], 0)
                                nc.gpsimd.memset(etb[:], 0)
                                nc.scalar.activation(eta[0:16, :], sta[0:16, :],
                                                     EXP, scale=INV,
                                                     bias=negbeta[0:16])
                                nc.scalar.activation(etb[0:16, :],
                                                     stb[0:16, 0:256], EXP,
                                                     scale=INV,
                                                     bias=negbeta[0:16])
                            else:
                                nc.scalar.activation(eta[:], sta[:], EXP,
                                                     scale=INV, bias=negbeta[:])
                                nc.scalar.activation(etb[:], stb[:, 0:256], EXP,
                                                     scale=INV, bias=negbeta[:])
                            if si == 0:
                                nc.vector.tensor_copy(out=Zacc[:, 0:1024],
                                                      in_=eta[:])
                                nc.vector.tensor_copy(out=Zacc[:, 1024:1280],
                                                      in_=etb[:])
                            else:
                                nc.vector.tensor_tensor(out=Zacc[:, 0:1024],
                                                        in0=Zacc[:, 0:1024],
                                                        in1=eta[:], op=ADD)
                                nc.vector.tensor_tensor(out=Zacc[:, 1024:1280],
                                                        in0=Zacc[:, 1024:1280],
                                                        in1=etb[:], op=ADD)
                            if pending is not None:
                                emit_pv(pending)
                            pending = (eta, etb, vtf[:, r, t, :],
                                       si == 0, si == nlast)
                        emit_pv(pending)

                    # ---------- epilogue ----------
                    with tc.tile_pool(name="ez", bufs=1, space="PSUM") as ezp:
                        zps = ezp.tile([1, ND], F32, tag="ez", space="PSUM")
                        for lo, hi in SL3:
                            nc.tensor.matmul(out=zps[:, lo:hi], lhsT=ones128[:],
                                             rhs=Zacc[:, lo:hi],
                                             start=True, stop=True)
                        zrow = sa.tile([1, ND], F32, tag="zrow")
                        nc.vector.tensor_scalar_add(out=zrow[:], in0=zps[:],
                                                    scalar1=1e-30)
                        rz = sa.tile([1, ND], F32, tag="rz")
                        nc.vector.reciprocal(out=rz[:], in_=zrow[:])
                        rq = sa.tile([1, ND], F32, tag="rq")
                        nc.vector.tensor_tensor(out=rq[:], in0=rz[:],
                                                in1=qmrow[:], op=MULT)
                        rbp = ezp.tile([128, ND], F32, tag="ez", space="PSUM")
                        for lo, hi in SL3:
                            nc.tensor.matmul(out=rbp[:, lo:hi], lhsT=ones1[:],
                                             rhs=rq[:, lo:hi],
                                             start=True, stop=True)
                        rb = sa.tile([128, ND], F32, tag="rb")
                        nc.vector.tensor_copy(out=rb[:], in_=rbp[:])
                        wn = sa.tile([128, ND], F32, tag="wn")
                        nc.vector.tensor_tensor(out=wn[:, 0:1024], in0=wta[:],
                                                in1=rb[:, 0:1024], op=MULT)
                        nc.vector.tensor_tensor(out=wn[:, 1024:1280], in0=wtb[:],
                                                in1=rb[:, 1024:1280], op=MULT)
                        acc = sa.tile([128, 1], F32, tag="acc")
                        nc.vector.reduce_sum(acc[:], wn[:],
                                             axis=mybir.AxisListType.X)
                        nc.sync.dma_start(out=accb[:], in_=acc[:])
                        nc.gpsimd.collective_compute(
                            "AllReduce", ADD, replica_groups=rg,
                            ins=[accb[:].opt()], outs=[accr[:].opt()],
                        )
                        accg = sa.tile([128, 1], F32, tag="accg")
                        nc.sync.dma_start(out=accg[:], in_=accr[:])
                        aggc = sa.tile([128, 1], F32, tag="aggc")
                        nc.scalar.activation(aggc[:], accg[:], IDENT,
                                             scale=1.0 / NREAL, bias=vgv[:])

                        # ---------- tiny MLP head (replicated) ----------
                        hd = ezp.tile([128, 512], F32, tag="hd", space="PSUM")
                        nc.tensor.matmul(out=hd[:, 0:1], lhsT=wo[:], rhs=aggc[:],
                                         start=True, stop=True)
                        state = sa.tile([128, 1], F32, tag="state")
                        nc.scalar.activation(state[:], hd[:, 0:1], IDENT, bias=bo[:])
                        hd2 = ezp.tile([128, 512], F32, tag="hd", space="PSUM")
                        nc.tensor.matmul(out=hd2[:64, 0:1], lhsT=wf1[:], rhs=state[:],
                                         start=True, stop=True)
                        x1 = sa.tile([64, 1], F32, tag="x1")
                        nc.scalar.activation(x1[:], hd2[:64, 0:1], RELU, bias=bf1[:])
                        hd3 = ezp.tile([128, 512], F32, tag="hd", space="PSUM")
                        nc.tensor.matmul(out=hd3[:32, 0:1], lhsT=wf2[:], rhs=x1[:],
                                         start=True, stop=True)
                        x2 = sa.tile([32, 1], F32, tag="x2")
                        nc.scalar.activation(x2[:], hd3[:32, 0:1], RELU, bias=bf2[:])
                        hd4 = ezp.tile([128, 512], F32, tag="hd", space="PSUM")
                        nc.tensor.matmul(out=hd4[:, 0:1], lhsT=wf3[:], rhs=x2[:],
                                         start=True, stop=True)
                        lg = sa.tile([128, 1], F32, tag="lg")
                        nc.scalar.activation(lg[:], hd4[:, 0:1], IDENT, bias=bf3[:])
                        hd5 = ezp.tile([128, 512], F32, tag="hd", space="PSUM")
                        nc.tensor.transpose(out=hd5[:1, 0:128], in_=lg[:],
                                            identity=ident[:])
                        er = sa.tile([1, 128], F32, tag="er")
                        zf = sa.tile([1, 1], F32, tag="zf")
                        nc.scalar.activation(er[:], hd5[:1, 0:128], EXP,
                                             accum_out=zf[:])
                        rzf = sa.tile([1, 1], F32, tag="rzf")
                        nc.vector.reciprocal(rzf[:], zf[:])
                        orow = sa.tile([1, 128], F32, tag="orow")
                        nc.vector.tensor_scalar(out=orow[:], in0=er[:],
                                                scalar1=rzf[:], scalar2=None,
                                                op0=MULT)
                        nc.sync.dma_start(out=out_t[:], in_=orow[:])

    nc.compile()
    return nc


def _get_nc():
    phase = int(os.environ.get("K_PHASE", "9"))
    key = ("nc", phase)
    if key not in _NC_CACHE:
        _NC_CACHE[key] = _build(phase)
    return _NC_CACHE[key]


def _prep_in_maps(inputs):
    f32 = np.float32
    x = np.asarray(inputs["node_features"], f32)
    g = np.asarray(inputs["global_info"], f32)
    ei = np.asarray(inputs["edge_index"])
    src = np.asarray(ei[0], np.int64)
    dst = np.asarray(ei[1], np.int64)

    xp = np.zeros((NP, D), f32)
    xp[:NREAL] = x
    xb = xp.astype(NP_BF16)
    x_tiled = np.ascontiguousarray(xb.reshape(SB, 128, D).transpose(1, 0, 2))

    qgv = (np.asarray(inputs["bQ"], f32)
           + (g @ np.asarray(inputs["WQg"], f32))[0]
           + np.asarray(inputs["bQg"], f32))
    vgv = (np.asarray(inputs["bV"], f32)
           + (g @ np.asarray(inputs["WVg"], f32))[0]
           + np.asarray(inputs["bVg"], f32))

    # host-side shift fit: c_q ~= Q_q . u + beta tracks the per-row max of
    # the (unshifted) scores so exp stays in fp32/bf16 range on device
    adj = sp.csr_matrix((np.ones(src.shape[0], f32), (dst, src)),
                        shape=(NREAL, NREAL))
    W1r = np.asarray(inputs["W1_root"], f32)
    W1l = np.asarray(inputs["W1_rel"], f32)
    W2r = np.asarray(inputs["W2_root"], f32)
    W2l = np.asarray(inputs["W2_rel"], f32)
    h_h = np.maximum(x @ W1r + (adj @ x) @ W1l + np.asarray(inputs["b1"], f32), 0)
    ne_h = h_h @ W2r + (adj @ h_h) @ W2l + np.asarray(inputs["b2"], f32)
    Q_h = ne_h @ np.asarray(inputs["WQ"], f32) + qgv
    K_h = ne_h @ np.asarray(inputs["WK"], f32)          # device K has no bias
    rowmax_sub = ((Q_h @ K_h[::16].T) * INV).max(axis=1)
    Afit = np.hstack([Q_h, np.ones((NREAL, 1), f32)]).astype(np.float64)
    yfit = rowmax_sub.astype(np.float64)
    AtA = Afit.T @ Afit
    lam = 1e-4 * np.mean(np.diag(AtA)[:D])
    sol = np.linalg.solve(AtA + lam * np.eye(D + 1), Afit.T @ yfit)
    resid = rowmax_sub - (Afit @ sol).astype(f32)
    delta = float(resid.max()) - 8.0
    u = sol[:D].astype(f32)
    beta = float(sol[D]) + delta
    negv = (-(u / INV)).reshape(D, 1).astype(f32)
    negbeta = np.full((D, 1), -beta, f32)

    def bf(name):
        return np.ascontiguousarray(np.asarray(inputs[name], f32).astype(NP_BF16))

    shared = {
        "w1r": bf("W1_root"), "w1l": bf("W1_rel"),
        "w2r": bf("W2_root"), "w2l": bf("W2_rel"),
        "wq": bf("WQ"), "wk": bf("WK"), "wv": bf("WV"),
        "b1": np.asarray(inputs["b1"], f32).reshape(D, 1),
        "b2": np.asarray(inputs["b2"], f32).reshape(D, 1),
        "qgv": qgv.reshape(D, 1).copy(), "vgv": vgv.reshape(D, 1).copy(),
        "negv": negv, "negbeta": negbeta,
        "wo": np.asarray(inputs["Wo"], f32),
        "wf1": np.asarray(inputs["Wfc1"], f32),
        "wf2": np.asarray(inputs["Wfc2"], f32),
        "wf3": np.asarray(inputs["Wfc3"], f32),
        "bo": np.asarray(inputs["bo"], f32).reshape(D, 1),
        "bf1": np.asarray(inputs["bfc1"], f32).reshape(64, 1),
        "bf2": np.asarray(inputs["bfc2"], f32).reshape(32, 1),
        "bf3": np.asarray(inputs["bfc3"], f32).reshape(D, 1),
        "x_tiled": x_tiled,
    }

    core_of = dst // ND
    in_maps = []
    nodes = np.arange(NP)
    for c in range(NCORES):
        m = core_of == c
        A = np.zeros((NP, ND), f32)
        np.add.at(A, (src[m], dst[m] - ND * c), 1.0)
        Ac = np.ascontiguousarray(
            A.reshape(SB, 128, ND).transpose(1, 0, 2)
        ).astype(NP_FP8)
        xTmc = np.ascontiguousarray(xb[ND * c : ND * (c + 1)].T)
        qm = (nodes[ND * c : ND * (c + 1)] < NREAL).astype(f32)
        in_maps.append({**shared, "a_cnt": Ac, "xT_mine": xTmc,
                        "qmask_row": qm.reshape(1, ND).copy()})
    return in_maps


def kernel(**inputs):
    nc = _get_nc()
    in_maps = _prep_in_maps(inputs)
    res = run_bass_kernel_spmd(nc, in_maps, core_ids=list(range(NCORES)))
    return np.asarray(res.results[0]["out"], np.float32)
